# revision 2
# baseline (speedup 1.0000x reference)
"""MendGraph kernel for 8 Trainium2 NeuronCores.

Computes (fill_feats, adj) of the reference:
  fill_feats = concat(org_feats, gen_feats.reshape(-1, F))          [16000, 256]
  A = scatter-add(org_edges) + scatter(mended edges)                [16000, 16000]
  A = max(A, A.T); A += I; adj = A / rowsum(A)

Sharding: 1D row partition of the [L, L] adjacency. Each core owns 500 "top"
rows (the original-node block, rows 500k..500k+500) and 1500 "bottom" rows
(the generated-node block, rows 4000+1500k..+1500), so the edge-scatter work
is balanced while every core streams the same 128 MB of output.

Device does all dense materialization and all value math (duplicate-summing
scatter via one-hot matmuls into PSUM, symmetrize max, row-sum, reciprocal,
scaling, mask values). Host only reorders edge indices into per-core buckets
and computes flat scatter offsets.
"""
import sys
sys.path.insert(0, "/opt/trn_rl_repo")

import numpy as np
from contextlib import ExitStack

import concourse.bass as bass
import concourse.bacc as bacc
import concourse.mybir as mybir
import concourse.tile as tile
from concourse.bass_utils import run_bass_kernel_spmd

# problem constants (hardcoded per harness contract)
N_ORG = 4000
NUM_PRED = 3
FEAT = 256
N_EDGES = 64000
L = N_ORG * (1 + NUM_PRED)          # 16000
NCORES = 8
TOP_PER_CORE = N_ORG // NCORES      # 500
BOT_PER_CORE = (L - N_ORG) // NCORES  # 1500
P = 125                             # tile height (partition rows used)
RT = TOP_PER_CORE // P              # 4 top tiles per core
BT = BOT_PER_CORE // P              # 12 bottom tiles per core
CT = 8                              # column tiles over the [*, 0:4000] block
CW = N_ORG // CT                    # 500 cols per column tile
CHUNK = 128                         # edge entries per one-hot matmul

f32 = mybir.dt.float32
i32 = mybir.dt.int32

_PROGRAM_CACHE = {}


def _build_program(nch):
    """Build the SPMD Bass program. nch[rt][ct][d] = chunk count (same on all cores)."""
    slots_base = {}
    s = 0
    for rt in range(RT):
        for ct in range(CT):
            for d in range(2):
                slots_base[(rt, ct, d)] = s
                s += nch[rt][ct][d]
    SLOTS = s

    nc = bacc.Bacc("TRN2", target_bir_lowering=False)

    # per-core inputs
    ridx = nc.dram_tensor("ridx", [CHUNK, SLOTS], f32, kind="ExternalInput")
    cidx = nc.dram_tensor("cidx", [CHUNK, SLOTS], f32, kind="ExternalInput")
    deg_in = nc.dram_tensor("deg_in", [P, RT], f32, kind="ExternalInput")
    soff_top = nc.dram_tensor("soff_top", [P, RT], i32, kind="ExternalInput")
    doff_top = nc.dram_tensor("doff_top", [P, RT], i32, kind="ExternalInput")
    sboff = nc.dram_tensor("sboff", [P, BT], i32, kind="ExternalInput")
    dgoff = nc.dram_tensor("dgoff", [P, BT], i32, kind="ExternalInput")
    jb_in = nc.dram_tensor("jb_in", [P, BT], f32, kind="ExternalInput")
    degb_in = nc.dram_tensor("degb_in", [P, BT], f32, kind="ExternalInput")
    feat_top_in = nc.dram_tensor("feat_top_in", [TOP_PER_CORE, FEAT], f32, kind="ExternalInput")
    feat_bot_in = nc.dram_tensor("feat_bot_in", [BOT_PER_CORE, FEAT], f32, kind="ExternalInput")

    # per-core outputs (flat; host reassembles)
    out_top = nc.dram_tensor("out_top", [TOP_PER_CORE * L], f32, kind="ExternalOutput")
    out_bot = nc.dram_tensor("out_bot", [BOT_PER_CORE * L], f32, kind="ExternalOutput")
    feat_top = nc.dram_tensor("feat_top", [TOP_PER_CORE, FEAT], f32, kind="ExternalOutput")
    feat_bot = nc.dram_tensor("feat_bot", [BOT_PER_CORE, FEAT], f32, kind="ExternalOutput")

    vtop = out_top[:].rearrange("(r c) -> r c", c=L)
    vbot = out_bot[:].rearrange("(r c) -> r c", c=L)
    ftop = out_top[:].rearrange("(a b) -> a b", b=1)
    fbot = out_bot[:].rearrange("(a b) -> a b", b=1)

    with tile.TileContext(nc) as tc, ExitStack() as ctx:
        io = ctx.enter_context(tc.tile_pool(name="io", bufs=1))
        work = ctx.enter_context(tc.tile_pool(name="work", bufs=2))
        oh = ctx.enter_context(tc.tile_pool(name="oh", bufs=3))
        sm = ctx.enter_context(tc.tile_pool(name="sm", bufs=2))
        psum = ctx.enter_context(tc.tile_pool(name="psum", bufs=2, space="PSUM"))

        # persistent small inputs
        ridx_t = io.tile([CHUNK, SLOTS], f32)
        cidx_t = io.tile([CHUNK, SLOTS], f32)
        deg_t = io.tile([P, RT], f32)
        soff_t = io.tile([P, RT], i32)
        doff_t = io.tile([P, RT], i32)
        sboff_t = io.tile([P, BT], i32)
        dgoff_t = io.tile([P, BT], i32)
        jb_t = io.tile([P, BT], f32)
        degb_t = io.tile([P, BT], f32)
        nc.sync.dma_start(ridx_t[:], ridx[:])
        nc.sync.dma_start(cidx_t[:], cidx[:])
        nc.sync.dma_start(deg_t[:], deg_in[:])
        nc.sync.dma_start(soff_t[:], soff_top[:])
        nc.sync.dma_start(doff_t[:], doff_top[:])
        nc.sync.dma_start(sboff_t[:], sboff[:])
        nc.sync.dma_start(dgoff_t[:], dgoff[:])
        nc.sync.dma_start(jb_t[:], jb_in[:])
        nc.sync.dma_start(degb_t[:], degb_in[:])

        # iotas (f32 values are small integers -> exact)
        iota_r_i = io.tile([CHUNK, P], i32)
        iota_c_i = io.tile([CHUNK, CW], i32)
        nc.gpsimd.iota(iota_r_i[:], pattern=[[1, P]], base=0, channel_multiplier=0)
        nc.gpsimd.iota(iota_c_i[:], pattern=[[1, CW]], base=0, channel_multiplier=0)
        iota_r = io.tile([CHUNK, P], f32)
        iota_c = io.tile([CHUNK, CW], f32)
        nc.vector.tensor_copy(iota_r[:], iota_r_i[:])
        nc.vector.tensor_copy(iota_c[:], iota_c_i[:])

        # zero source tile (read-only after memset)
        z = io.tile([128, 8000], f32)
        nc.vector.memset(z[:], 0.0)

        # ---- bottom block: zeros + staircase/diag values via indirect runs ----
        for bt in range(BT):
            r0 = bt * P
            nc.sync.dma_start(vbot[r0:r0 + P, 0:8000], z[:P, :])
            nc.sync.dma_start(vbot[r0:r0 + P, 8000:16000], z[:P, :])
            # mask = (jb < degb); stair = 0.5*mask; diag = 1 - 0.5*mask
            mask = sm.tile([P, 1], f32, tag="maskb")
            nc.vector.tensor_tensor(mask[:], jb_t[:, bt:bt + 1], degb_t[:, bt:bt + 1],
                                    op=mybir.AluOpType.is_lt)
            stair = sm.tile([P, 1], f32, tag="stairb")
            nc.vector.tensor_scalar(out=stair[:], in0=mask[:], scalar1=0.5, scalar2=None,
                                    op0=mybir.AluOpType.mult)
            diag = sm.tile([P, 1], f32, tag="diagb")
            nc.vector.tensor_scalar(out=diag[:], in0=mask[:], scalar1=-0.5, scalar2=1.0,
                                    op0=mybir.AluOpType.mult, op1=mybir.AluOpType.add)
            nc.gpsimd.indirect_dma_start(
                out=fbot, out_offset=bass.IndirectOffsetOnAxis(ap=sboff_t[:, bt:bt + 1], axis=0),
                in_=stair[:], in_offset=None)
            nc.gpsimd.indirect_dma_start(
                out=fbot, out_offset=bass.IndirectOffsetOnAxis(ap=dgoff_t[:, bt:bt + 1], axis=0),
                in_=diag[:], in_offset=None)

        # ---- top block ----
        for rt in range(RT):
            r0 = rt * P
            left = work.tile([P, N_ORG], f32, tag="left")
            for ct in range(CT):
                pF = psum.tile([P, CW], f32, tag="pF", space="PSUM")
                pR = psum.tile([P, CW], f32, tag="pR", space="PSUM")
                for d, pt in ((0, pF), (1, pR)):
                    n = nch[rt][ct][d]
                    base = slots_base[(rt, ct, d)]
                    for i in range(n):
                        sL = base + i
                        r1 = oh.tile([CHUNK, P], f32, tag="r1")
                        c1 = oh.tile([CHUNK, CW], f32, tag="c1")
                        nc.vector.tensor_tensor(
                            r1[:], iota_r[:], ridx_t[:, sL:sL + 1].to_broadcast([CHUNK, P]),
                            op=mybir.AluOpType.is_equal)
                        nc.vector.tensor_tensor(
                            c1[:], iota_c[:], cidx_t[:, sL:sL + 1].to_broadcast([CHUNK, CW]),
                            op=mybir.AluOpType.is_equal)
                        nc.tensor.matmul(pt[:], lhsT=r1[:], rhs=c1[:],
                                         start=(i == 0), stop=(i == n - 1))
                lslice = left[:, ct * CW:(ct + 1) * CW]
                nc.vector.tensor_copy(lslice, pF[:])
                nc.vector.tensor_tensor(lslice, lslice, pR[:], op=mybir.AluOpType.max)
            # rowsum = sum(left) + deg + 1 (self-loop)
            rs = sm.tile([P, 1], f32, tag="rs")
            nc.vector.tensor_reduce(rs[:], left[:], op=mybir.AluOpType.add,
                                    axis=mybir.AxisListType.X)
            nc.vector.tensor_tensor(rs[:], rs[:], deg_t[:, rt:rt + 1], op=mybir.AluOpType.add)
            rinv = sm.tile([P, 1], f32, tag="rinv")
            nc.vector.tensor_scalar(out=rs[:], in0=rs[:], scalar1=1.0, scalar2=None,
                                    op0=mybir.AluOpType.add)
            nc.vector.reciprocal(rinv[:], rs[:])
            nc.vector.tensor_tensor(left[:], left[:], rinv[:].to_broadcast([P, N_ORG]),
                                    op=mybir.AluOpType.mult)
            # dense writes: left block + zero tail
            nc.sync.dma_start(vtop[r0:r0 + P, 0:N_ORG], left[:])
            nc.sync.dma_start(vtop[r0:r0 + P, 4000:12000], z[:P, :])
            nc.sync.dma_start(vtop[r0:r0 + P, 12000:16000], z[:P, :4000])
            # diag self-loop: scatter-add rinv at (p, row_global) inside left block
            nc.gpsimd.indirect_dma_start(
                out=ftop, out_offset=bass.IndirectOffsetOnAxis(ap=doff_t[:, rt:rt + 1], axis=0),
                in_=rinv[:], in_offset=None, compute_op=mybir.AluOpType.add)
            # strip values [P, 3]: (deg > j) * rinv, 3-element contiguous runs
            sv = sm.tile([P, 3], f32, tag="sv")
            for j in range(3):
                g = sm.tile([P, 1], f32, tag="gj")
                nc.vector.tensor_scalar(out=g[:], in0=deg_t[:, rt:rt + 1], scalar1=float(j),
                                        scalar2=None, op0=mybir.AluOpType.is_gt)
                nc.vector.tensor_tensor(sv[:, j:j + 1], g[:], rinv[:], op=mybir.AluOpType.mult)
            nc.gpsimd.indirect_dma_start(
                out=ftop, out_offset=bass.IndirectOffsetOnAxis(ap=soff_t[:, rt:rt + 1], axis=0),
                in_=sv[:], in_offset=None)

        # ---- fill_feats passthrough ----
        fb_top = work.tile([P, 4 * FEAT], f32, tag="fbt")
        nc.sync.dma_start(fb_top[:], feat_top_in[:].rearrange("(p a) d -> p (a d)", p=P))
        nc.sync.dma_start(feat_top[:].rearrange("(p a) d -> p (a d)", p=P), fb_top[:])
        fb_bot = work.tile([P, 12 * FEAT], f32, tag="fbb")
        nc.sync.dma_start(fb_bot[:], feat_bot_in[:].rearrange("(p a) d -> p (a d)", p=P))
        nc.sync.dma_start(feat_bot[:].rearrange("(p a) d -> p (a d)", p=P), fb_bot[:])

    nc.compile()
    return nc


def _prepare_inputs(org_feats, org_edges, pred_missing, gen_feats):
    """Host-side index prep: bucket edges per (core, row-tile, col-tile, dir),
    compute scatter offsets. Returns (nch, in_maps)."""
    org_feats = np.ascontiguousarray(org_feats, dtype=np.float32)
    org_edges = np.ascontiguousarray(org_edges, dtype=np.int64)
    pred_missing = np.asarray(pred_missing, dtype=np.int64)
    gen_flat = np.ascontiguousarray(gen_feats, dtype=np.float32).reshape(-1, FEAT)

    deg = np.clip(pred_missing, 0, NUM_PRED).astype(np.float32)  # [N_ORG]

    er, ec = org_edges[:, 0], org_edges[:, 1]
    rows = np.concatenate([er, ec])          # fwd rows, then rev rows
    cols = np.concatenate([ec, er])
    dirs = np.concatenate([np.zeros(N_EDGES, np.int64), np.ones(N_EDGES, np.int64)])

    core = rows // TOP_PER_CORE
    rloc = rows % TOP_PER_CORE
    rt = rloc // P
    pl = rloc % P
    ct = cols // CW
    lc = cols % CW

    # stable sort by (core, rt, ct, dir)
    key = ((core * RT + rt) * CT + ct) * 2 + dirs
    order = np.argsort(key, kind="stable")
    ks = key[order]
    # position within bucket
    bucket_start = np.searchsorted(ks, np.arange(NCORES * RT * CT * 2))
    counts = np.diff(np.append(bucket_start, len(ks)))
    pos = np.arange(len(ks)) - bucket_start[ks]

    # chunk counts shared across cores per (rt, ct, d)
    cnt = counts.reshape(NCORES, RT, CT, 2)
    nch_arr = np.maximum(1, (cnt.max(axis=0) + CHUNK - 1) // CHUNK)  # [RT, CT, 2]
    nch = nch_arr.tolist()
    slots_base = np.zeros((RT, CT, 2), np.int64)
    s = 0
    for a in range(RT):
        for b in range(CT):
            for d in range(2):
                slots_base[a, b, d] = s
                s += nch_arr[a, b, d]
    SLOTS = s

    # fill per-core index arrays (999 = no-match padding)
    ridx_all = np.full((NCORES, CHUNK, SLOTS), 999.0, np.float32)
    cidx_all = np.zeros((NCORES, CHUNK, SLOTS), np.float32)
    oc = core[order]; ort = rt[order]; oct_ = ct[order]; od = dirs[order]
    opl = pl[order]; olc = lc[order]
    slot = slots_base[ort, oct_, od] + pos // CHUNK
    q = pos % CHUNK
    ridx_all[oc, q, slot] = opl
    cidx_all[oc, q, slot] = olc

    in_maps = []
    for k in range(NCORES):
        i_loc = np.arange(TOP_PER_CORE, dtype=np.int64)
        i_glob = k * TOP_PER_CORE + i_loc
        p_t = i_loc % P
        rt_t = i_loc // P
        deg_in = np.zeros((P, RT), np.float32)
        soff_top = np.zeros((P, RT), np.int32)
        doff_top = np.zeros((P, RT), np.int32)
        deg_in[p_t, rt_t] = deg[i_glob]
        soff_top[p_t, rt_t] = i_loc * L + N_ORG + 3 * i_glob
        doff_top[p_t, rt_t] = i_loc * L + i_glob

        m_loc = np.arange(BOT_PER_CORE, dtype=np.int64)
        m_glob = k * BOT_PER_CORE + m_loc
        p_b = m_loc % P
        bt_b = m_loc // P
        sboff = np.zeros((P, BT), np.int32)
        dgoff = np.zeros((P, BT), np.int32)
        jb = np.zeros((P, BT), np.float32)
        degb = np.zeros((P, BT), np.float32)
        sboff[p_b, bt_b] = m_loc * L + m_glob // NUM_PRED
        dgoff[p_b, bt_b] = m_loc * L + N_ORG + m_glob
        jb[p_b, bt_b] = m_glob % NUM_PRED
        degb[p_b, bt_b] = deg[m_glob // NUM_PRED]

        in_maps.append({
            "ridx": ridx_all[k], "cidx": cidx_all[k],
            "deg_in": deg_in, "soff_top": soff_top, "doff_top": doff_top,
            "sboff": sboff, "dgoff": dgoff, "jb_in": jb, "degb_in": degb,
            "feat_top_in": org_feats[k * TOP_PER_CORE:(k + 1) * TOP_PER_CORE],
            "feat_bot_in": gen_flat[k * BOT_PER_CORE:(k + 1) * BOT_PER_CORE],
        })
    return tuple(map(tuple, (tuple(map(tuple, r)) for r in nch))), in_maps


def kernel(org_feats, org_edges, pred_missing, gen_feats, _trace=False):
    nch, in_maps = _prepare_inputs(org_feats, org_edges, pred_missing, gen_feats)
    if nch not in _PROGRAM_CACHE:
        _PROGRAM_CACHE[nch] = _build_program([[list(c) for c in r] for r in nch])
    nc = _PROGRAM_CACHE[nch]

    res = run_bass_kernel_spmd(nc, in_maps, core_ids=list(range(NCORES)), trace=_trace)

    adj = np.empty((L, L), np.float32)
    fill = np.empty((L, FEAT), np.float32)
    for k in range(NCORES):
        r = res.results[k]
        adj[k * TOP_PER_CORE:(k + 1) * TOP_PER_CORE] = \
            r["out_top"].reshape(TOP_PER_CORE, L)
        adj[N_ORG + k * BOT_PER_CORE:N_ORG + (k + 1) * BOT_PER_CORE] = \
            r["out_bot"].reshape(BOT_PER_CORE, L)
        fill[k * TOP_PER_CORE:(k + 1) * TOP_PER_CORE] = r["feat_top"]
        fill[N_ORG + k * BOT_PER_CORE:N_ORG + (k + 1) * BOT_PER_CORE] = r["feat_bot"]
    if _trace:
        return (fill, adj), res
    return fill, adj


# revision 7
# speedup vs baseline: 1.4369x; 1.4369x over previous
"""MendGraph kernel for 8 Trainium2 NeuronCores.

Computes (fill_feats, adj) of the reference:
  fill_feats = concat(org_feats, gen_feats.reshape(-1, F))          [16000, 256]
  A = scatter-add(org_edges) + scatter(mended edges)                [16000, 16000]
  A = max(A, A.T); A += I; adj = A / rowsum(A)

Sharding: 1D row partition of the [L, L] adjacency. Each core owns 500 "top"
rows (the original-node block, rows 500k..500k+500) and 1500 "bottom" rows
(the generated-node block, rows 4000+1500k..+1500), so the edge-scatter work
is balanced while every core streams the same 128 MB of output.

Device does all dense materialization and all value math (duplicate-summing
scatter via one-hot matmuls into PSUM, symmetrize max, row-sum, reciprocal,
scaling, mask values). Host only reorders edge indices into per-core buckets
and computes flat scatter offsets.
"""
import sys
sys.path.insert(0, "/opt/trn_rl_repo")

import numpy as np
from contextlib import ExitStack

import concourse.bass as bass
import concourse.bacc as bacc
import concourse.mybir as mybir
import concourse.tile as tile
from concourse.tile import add_dep_helper
from concourse.bass_utils import run_bass_kernel_spmd

# problem constants (hardcoded per harness contract)
N_ORG = 4000
NUM_PRED = 3
FEAT = 256
N_EDGES = 64000
L = N_ORG * (1 + NUM_PRED)          # 16000
NCORES = 8
TOP_PER_CORE = N_ORG // NCORES      # 500
BOT_PER_CORE = (L - N_ORG) // NCORES  # 1500
P = 125                             # tile height (partition rows used)
RT = TOP_PER_CORE // P              # 4 top tiles per core
BT = BOT_PER_CORE // P              # 12 bottom tiles per core
CT = 8                              # column tiles over the [*, 0:4000] block
CW = N_ORG // CT                    # 500 cols per column tile
CHUNK = 128                         # edge entries per one-hot matmul

f32 = mybir.dt.float32
i32 = mybir.dt.int32

_PROGRAM_CACHE = {}


def _build_program(nch):
    """Build the SPMD Bass program. nch[rt][ct][d] = chunk count (same on all cores)."""
    slots_base = {}
    s = 0
    for rt in range(RT):
        for ct in range(CT):
            for d in range(2):
                slots_base[(rt, ct, d)] = s
                s += nch[rt][ct][d]
    SLOTS = s

    nc = bacc.Bacc("TRN2", target_bir_lowering=False)

    # per-core inputs
    ridx = nc.dram_tensor("ridx", [CHUNK, SLOTS], f32, kind="ExternalInput")
    cidx = nc.dram_tensor("cidx", [CHUNK, SLOTS], f32, kind="ExternalInput")
    deg_in = nc.dram_tensor("deg_in", [P, RT], f32, kind="ExternalInput")
    soff_top = nc.dram_tensor("soff_top", [P, RT], i32, kind="ExternalInput")
    doff_top = nc.dram_tensor("doff_top", [P, RT], i32, kind="ExternalInput")
    sboff = nc.dram_tensor("sboff", [P, BT], i32, kind="ExternalInput")
    dgoff = nc.dram_tensor("dgoff", [P, BT], i32, kind="ExternalInput")
    jb_in = nc.dram_tensor("jb_in", [P, BT], f32, kind="ExternalInput")
    degb_in = nc.dram_tensor("degb_in", [P, BT], f32, kind="ExternalInput")
    feat_top_in = nc.dram_tensor("feat_top_in", [TOP_PER_CORE, FEAT], f32, kind="ExternalInput")
    feat_bot_in = nc.dram_tensor("feat_bot_in", [BOT_PER_CORE, FEAT], f32, kind="ExternalInput")

    # per-core outputs (flat; host reassembles)
    out_top = nc.dram_tensor("out_top", [TOP_PER_CORE * L], f32, kind="ExternalOutput")
    out_bot = nc.dram_tensor("out_bot", [BOT_PER_CORE * L], f32, kind="ExternalOutput")
    feat_top = nc.dram_tensor("feat_top", [TOP_PER_CORE, FEAT], f32, kind="ExternalOutput")
    feat_bot = nc.dram_tensor("feat_bot", [BOT_PER_CORE, FEAT], f32, kind="ExternalOutput")

    vtop = out_top[:].rearrange("(r c) -> r c", c=L)
    vbot = out_bot[:].rearrange("(r c) -> r c", c=L)
    ftop = out_top[:].rearrange("(a b) -> a b", b=1)
    fbot = out_bot[:].rearrange("(a b) -> a b", b=1)

    with tile.TileContext(nc) as tc, ExitStack() as ctx:
        io = ctx.enter_context(tc.tile_pool(name="io", bufs=1))
        work = ctx.enter_context(tc.tile_pool(name="work", bufs=2))
        oh = ctx.enter_context(tc.tile_pool(name="oh", bufs=3))
        sm = ctx.enter_context(tc.tile_pool(name="sm", bufs=2))
        psum = ctx.enter_context(tc.tile_pool(name="psum", bufs=2, space="PSUM"))

        # persistent small inputs
        ridx_t = io.tile([CHUNK, SLOTS], f32)
        cidx_t = io.tile([CHUNK, SLOTS], f32)
        deg_t = io.tile([P, RT], f32)
        soff_t = io.tile([P, RT], i32)
        doff_t = io.tile([P, RT], i32)
        sboff_t = io.tile([P, BT], i32)
        dgoff_t = io.tile([P, BT], i32)
        jb_t = io.tile([P, BT], f32)
        degb_t = io.tile([P, BT], f32)
        nc.sync.dma_start(ridx_t[:], ridx[:])
        nc.sync.dma_start(cidx_t[:], cidx[:])
        nc.sync.dma_start(deg_t[:], deg_in[:])
        nc.sync.dma_start(soff_t[:], soff_top[:])
        nc.sync.dma_start(doff_t[:], doff_top[:])
        nc.sync.dma_start(sboff_t[:], sboff[:])
        nc.sync.dma_start(dgoff_t[:], dgoff[:])
        nc.sync.dma_start(jb_t[:], jb_in[:])
        nc.sync.dma_start(degb_t[:], degb_in[:])

        # iotas (f32 values are small integers -> exact)
        iota_r_i = io.tile([CHUNK, P], i32)
        iota_c_i = io.tile([CHUNK, CW], i32)
        nc.gpsimd.iota(iota_r_i[:], pattern=[[1, P]], base=0, channel_multiplier=0)
        nc.gpsimd.iota(iota_c_i[:], pattern=[[1, CW]], base=0, channel_multiplier=0)
        iota_r = io.tile([CHUNK, P], f32)
        iota_c = io.tile([CHUNK, CW], f32)
        nc.vector.tensor_copy(iota_r[:], iota_r_i[:])
        nc.vector.tensor_copy(iota_c[:], iota_c_i[:])

        # zero source tile (read-only after memset)
        z = io.tile([128, 8000], f32)
        nc.vector.memset(z[:], 0.0)

        # manual completion semaphores: bulk writes inc them at data-landed;
        # deferred scatters wait on them (same-queue SWDGE issue order does NOT
        # imply landing order across the 16 per-engine rings).
        semb = [ctx.enter_context(nc.semaphore(f"semb{t}")) for t in range(BT)]
        semt = [ctx.enter_context(nc.semaphore(f"semt{t}")) for t in range(RT)]
        seml = [ctx.enter_context(nc.semaphore(f"seml{t}")) for t in range(RT)]

        # persistent value/offset stores for deferred scatters
        stair_all = io.tile([P, BT], f32)
        diag_all = io.tile([P, BT], f32)
        sv_all = io.tile([P, 3 * RT], f32)
        rinv_all = io.tile([P, RT], f32)

        zb_insts = {}
        zt_insts = {}
        zl_insts = {}

        def emit_bot_scatter(bt):
            sc0 = nc.gpsimd.indirect_dma_start(
                out=fbot, out_offset=bass.IndirectOffsetOnAxis(ap=sboff_t[:, bt:bt + 1], axis=0),
                in_=stair_all[:, bt:bt + 1], in_offset=None)
            sc0._wait_ge(semb[bt], 32)
            sc1 = nc.gpsimd.indirect_dma_start(
                out=fbot, out_offset=bass.IndirectOffsetOnAxis(ap=dgoff_t[:, bt:bt + 1], axis=0),
                in_=diag_all[:, bt:bt + 1], in_offset=None)
            sc1._wait_ge(semb[bt], 32)
            for zb in zb_insts[bt]:
                add_dep_helper(sc0.ins, zb.ins, sync=False, reason="order stair after zeros")
                add_dep_helper(sc1.ins, zb.ins, sync=False, reason="order diag after zeros")

        def emit_top_scatter(rt):
            scd = nc.gpsimd.indirect_dma_start(
                out=ftop, out_offset=bass.IndirectOffsetOnAxis(ap=doff_t[:, rt:rt + 1], axis=0),
                in_=rinv_all[:, rt:rt + 1], in_offset=None, compute_op=mybir.AluOpType.add)
            scd._wait_ge(seml[rt], 16)
            add_dep_helper(scd.ins, zl_insts[rt].ins, sync=False, reason="order diag-add after left")
            scs = nc.gpsimd.indirect_dma_start(
                out=ftop, out_offset=bass.IndirectOffsetOnAxis(ap=soff_t[:, rt:rt + 1], axis=0),
                in_=sv_all[:, 3 * rt:3 * rt + 3], in_offset=None)
            scs._wait_ge(semt[rt], 16)
            add_dep_helper(scs.ins, zt_insts[rt].ins, sync=False, reason="order strip after zeros")

        LAG = 2

        # ---- bottom block: zeros + deferred staircase/diag value scatters ----
        for bt in range(BT):
            r0 = bt * P
            zb0 = nc.gpsimd.dma_start(vbot[r0:r0 + P, 0:8000], z[:P, :])
            zb0.then_inc(semb[bt], 16)
            zb1 = nc.gpsimd.dma_start(vbot[r0:r0 + P, 8000:16000], z[:P, :])
            zb1.then_inc(semb[bt], 16)
            zb_insts[bt] = (zb0, zb1)
            # mask = (jb < degb); stair = 0.5*mask; diag = 1 - 0.5*mask
            mask = sm.tile([P, 1], f32, tag="maskb")
            nc.vector.tensor_tensor(mask[:], jb_t[:, bt:bt + 1], degb_t[:, bt:bt + 1],
                                    op=mybir.AluOpType.is_lt)
            nc.vector.tensor_scalar(out=stair_all[:, bt:bt + 1], in0=mask[:], scalar1=0.5,
                                    scalar2=None, op0=mybir.AluOpType.mult)
            nc.vector.tensor_scalar(out=diag_all[:, bt:bt + 1], in0=mask[:], scalar1=-0.5,
                                    scalar2=1.0, op0=mybir.AluOpType.mult,
                                    op1=mybir.AluOpType.add)
            if bt >= LAG:
                emit_bot_scatter(bt - LAG)

        # ---- top block ----
        for rt in range(RT):
            r0 = rt * P
            left = work.tile([P, N_ORG], f32, tag="left")
            for ct in range(CT):
                pF = psum.tile([P, CW], f32, tag="pF", space="PSUM")
                pR = psum.tile([P, CW], f32, tag="pR", space="PSUM")
                for d, pt in ((0, pF), (1, pR)):
                    n = nch[rt][ct][d]
                    base = slots_base[(rt, ct, d)]
                    for i in range(n):
                        sL = base + i
                        r1 = oh.tile([CHUNK, P], f32, tag="r1")
                        c1 = oh.tile([CHUNK, CW], f32, tag="c1")
                        nc.vector.tensor_tensor(
                            r1[:], iota_r[:], ridx_t[:, sL:sL + 1].to_broadcast([CHUNK, P]),
                            op=mybir.AluOpType.is_equal)
                        nc.vector.tensor_tensor(
                            c1[:], iota_c[:], cidx_t[:, sL:sL + 1].to_broadcast([CHUNK, CW]),
                            op=mybir.AluOpType.is_equal)
                        nc.tensor.matmul(pt[:], lhsT=r1[:], rhs=c1[:],
                                         start=(i == 0), stop=(i == n - 1))
                lslice = left[:, ct * CW:(ct + 1) * CW]
                nc.vector.tensor_copy(lslice, pF[:])
                nc.vector.tensor_tensor(lslice, lslice, pR[:], op=mybir.AluOpType.max)
            # rowsum = sum(left) + deg + 1 (self-loop added via scatter-add later)
            rs = sm.tile([P, 1], f32, tag="rs")
            nc.vector.tensor_reduce(rs[:], left[:], op=mybir.AluOpType.add,
                                    axis=mybir.AxisListType.X)
            nc.vector.tensor_tensor(rs[:], rs[:], deg_t[:, rt:rt + 1], op=mybir.AluOpType.add)
            rinv = sm.tile([P, 1], f32, tag="rinv")
            nc.vector.tensor_scalar(out=rs[:], in0=rs[:], scalar1=1.0, scalar2=None,
                                    op0=mybir.AluOpType.add)
            nc.vector.reciprocal(rinv[:], rs[:])
            nc.vector.tensor_copy(rinv_all[:, rt:rt + 1], rinv[:])
            nc.vector.tensor_tensor(left[:], left[:], rinv[:].to_broadcast([P, N_ORG]),
                                    op=mybir.AluOpType.mult)
            # strip values: (deg > j) * rinv
            for j in range(3):
                g = sm.tile([P, 1], f32, tag="gj")
                nc.vector.tensor_scalar(out=g[:], in0=deg_t[:, rt:rt + 1], scalar1=float(j),
                                        scalar2=None, op0=mybir.AluOpType.is_gt)
                nc.vector.tensor_tensor(sv_all[:, 3 * rt + j:3 * rt + j + 1], g[:], rinv[:],
                                        op=mybir.AluOpType.mult)
            # dense writes: left block + zero tail
            zl = nc.gpsimd.dma_start(vtop[r0:r0 + P, 0:N_ORG], left[:])
            zl.then_inc(seml[rt], 16)
            zl_insts[rt] = zl
            zt0 = nc.gpsimd.dma_start(vtop[r0:r0 + P, 4000:12000], z[:P, :])
            zt0.then_inc(semt[rt], 16)
            zt_insts[rt] = zt0
            nc.gpsimd.dma_start(vtop[r0:r0 + P, 12000:16000], z[:P, :4000])
            # flush deferred scatters with one-phase lag
            if rt < LAG:
                emit_bot_scatter(BT - LAG + rt)
            if rt >= 1:
                emit_top_scatter(rt - 1)

        # ---- fill_feats passthrough ----
        fb_top = work.tile([P, 4 * FEAT], f32, tag="fbt")
        nc.sync.dma_start(fb_top[:], feat_top_in[:].rearrange("(p a) d -> p (a d)", p=P))
        nc.gpsimd.dma_start(feat_top[:].rearrange("(p a) d -> p (a d)", p=P), fb_top[:])
        fb_bot = work.tile([P, 12 * FEAT], f32, tag="fbb")
        nc.sync.dma_start(fb_bot[:], feat_bot_in[:].rearrange("(p a) d -> p (a d)", p=P))
        nc.gpsimd.dma_start(feat_bot[:].rearrange("(p a) d -> p (a d)", p=P), fb_bot[:])

        emit_top_scatter(RT - 1)

    nc.compile()
    return nc


def _prepare_inputs(org_feats, org_edges, pred_missing, gen_feats):
    """Host-side index prep: bucket edges per (core, row-tile, col-tile, dir),
    compute scatter offsets. Returns (nch, in_maps)."""
    org_feats = np.ascontiguousarray(org_feats, dtype=np.float32)
    org_edges = np.ascontiguousarray(org_edges, dtype=np.int64)
    pred_missing = np.asarray(pred_missing, dtype=np.int64)
    gen_flat = np.ascontiguousarray(gen_feats, dtype=np.float32).reshape(-1, FEAT)

    deg = np.clip(pred_missing, 0, NUM_PRED).astype(np.float32)  # [N_ORG]

    er, ec = org_edges[:, 0], org_edges[:, 1]
    rows = np.concatenate([er, ec])          # fwd rows, then rev rows
    cols = np.concatenate([ec, er])
    dirs = np.concatenate([np.zeros(N_EDGES, np.int64), np.ones(N_EDGES, np.int64)])

    core = rows // TOP_PER_CORE
    rloc = rows % TOP_PER_CORE
    rt = rloc // P
    pl = rloc % P
    ct = cols // CW
    lc = cols % CW

    # stable sort by (core, rt, ct, dir)
    key = ((core * RT + rt) * CT + ct) * 2 + dirs
    order = np.argsort(key, kind="stable")
    ks = key[order]
    # position within bucket
    bucket_start = np.searchsorted(ks, np.arange(NCORES * RT * CT * 2))
    counts = np.diff(np.append(bucket_start, len(ks)))
    pos = np.arange(len(ks)) - bucket_start[ks]

    # chunk counts shared across cores per (rt, ct, d)
    cnt = counts.reshape(NCORES, RT, CT, 2)
    nch_arr = np.maximum(1, (cnt.max(axis=0) + CHUNK - 1) // CHUNK)  # [RT, CT, 2]
    nch = nch_arr.tolist()
    slots_base = np.zeros((RT, CT, 2), np.int64)
    s = 0
    for a in range(RT):
        for b in range(CT):
            for d in range(2):
                slots_base[a, b, d] = s
                s += nch_arr[a, b, d]
    SLOTS = s

    # fill per-core index arrays (999 = no-match padding)
    ridx_all = np.full((NCORES, CHUNK, SLOTS), 999.0, np.float32)
    cidx_all = np.zeros((NCORES, CHUNK, SLOTS), np.float32)
    oc = core[order]; ort = rt[order]; oct_ = ct[order]; od = dirs[order]
    opl = pl[order]; olc = lc[order]
    slot = slots_base[ort, oct_, od] + pos // CHUNK
    q = pos % CHUNK
    ridx_all[oc, q, slot] = opl
    cidx_all[oc, q, slot] = olc

    in_maps = []
    for k in range(NCORES):
        i_loc = np.arange(TOP_PER_CORE, dtype=np.int64)
        i_glob = k * TOP_PER_CORE + i_loc
        p_t = i_loc % P
        rt_t = i_loc // P
        deg_in = np.zeros((P, RT), np.float32)
        soff_top = np.zeros((P, RT), np.int32)
        doff_top = np.zeros((P, RT), np.int32)
        deg_in[p_t, rt_t] = deg[i_glob]
        soff_top[p_t, rt_t] = i_loc * L + N_ORG + 3 * i_glob
        doff_top[p_t, rt_t] = i_loc * L + i_glob

        m_loc = np.arange(BOT_PER_CORE, dtype=np.int64)
        m_glob = k * BOT_PER_CORE + m_loc
        p_b = m_loc % P
        bt_b = m_loc // P
        sboff = np.zeros((P, BT), np.int32)
        dgoff = np.zeros((P, BT), np.int32)
        jb = np.zeros((P, BT), np.float32)
        degb = np.zeros((P, BT), np.float32)
        sboff[p_b, bt_b] = m_loc * L + m_glob // NUM_PRED
        dgoff[p_b, bt_b] = m_loc * L + N_ORG + m_glob
        jb[p_b, bt_b] = m_glob % NUM_PRED
        degb[p_b, bt_b] = deg[m_glob // NUM_PRED]

        in_maps.append({
            "ridx": ridx_all[k], "cidx": cidx_all[k],
            "deg_in": deg_in, "soff_top": soff_top, "doff_top": doff_top,
            "sboff": sboff, "dgoff": dgoff, "jb_in": jb, "degb_in": degb,
            "feat_top_in": org_feats[k * TOP_PER_CORE:(k + 1) * TOP_PER_CORE],
            "feat_bot_in": gen_flat[k * BOT_PER_CORE:(k + 1) * BOT_PER_CORE],
        })
    return tuple(map(tuple, (tuple(map(tuple, r)) for r in nch))), in_maps


def kernel(org_feats, org_edges, pred_missing, gen_feats, _trace=False):
    nch, in_maps = _prepare_inputs(org_feats, org_edges, pred_missing, gen_feats)
    if nch not in _PROGRAM_CACHE:
        _PROGRAM_CACHE[nch] = _build_program([[list(c) for c in r] for r in nch])
    nc = _PROGRAM_CACHE[nch]

    res = run_bass_kernel_spmd(nc, in_maps, core_ids=list(range(NCORES)), trace=_trace)

    adj = np.empty((L, L), np.float32)
    fill = np.empty((L, FEAT), np.float32)
    for k in range(NCORES):
        r = res.results[k]
        adj[k * TOP_PER_CORE:(k + 1) * TOP_PER_CORE] = \
            r["out_top"].reshape(TOP_PER_CORE, L)
        adj[N_ORG + k * BOT_PER_CORE:N_ORG + (k + 1) * BOT_PER_CORE] = \
            r["out_bot"].reshape(BOT_PER_CORE, L)
        fill[k * TOP_PER_CORE:(k + 1) * TOP_PER_CORE] = r["feat_top"]
        fill[N_ORG + k * BOT_PER_CORE:N_ORG + (k + 1) * BOT_PER_CORE] = r["feat_bot"]
    if _trace:
        return (fill, adj), res
    return fill, adj
